# revision 9
# baseline (speedup 1.0000x reference)
"""Trainium2 Bass kernel for nn_ConvPlus1d (dense_cnn).

Algorithm (mathematically identical to the reference, derived analytically):

  The reference synthesizes per-sample conv weights:
      kern[b]   = mean_L(depthwise_conv(x))        -> [B, C_IN, K]
      w_in[b]   = W_in @ kern[b]                   -> [B, C_IN, K]
      w_out[b]  = <W_out, kern[b]>                 -> [B, C_OUT]
      bias[b]   = <W_bias, kern[b]>                -> [B, C_OUT]
      weight[b, o, c, k] = w_in[b, c, k] * w_out[b, o]     (rank-1!)
      y[b] = conv1d(x[b], weight[b], pad=1) + bias[b]

  Exact simplifications:
  1) mean over L of a pad-1 depthwise conv only needs per-channel sums and
     the first/last elements:  sum_l xpad[c, l+t] = {S-E, S, S-F}[t]
     so kern / w_in / w_out / bias are LINEAR in (S, E, F), with
     coefficient matrices precomputed on the host from maker params.
  2) The per-sample conv weight is rank-1 across (o) x (c,k).

  Device program per sample (data-parallel over batch, 4 samples/core):
      x (bf16) lands in SBUF partitions 0-63; a shifted copy (one column
      left) is DMA'd into partitions 64-127.  The 3-tap conv then needs
      only TWO matmuls per 512-col tile: a 128-contract matmul computes
      taps 0+1 together (stationary [W0; W1]), a 64-contract matmul adds
      tap 2.  Stats -> params synthesis runs in fp32r (1 cyc/row).
      PSUM -> SBUF eviction adds the bias (ACT/DVE alternating); stores
      stream out in 1024-col chunks.

Sharding: batch 32 -> 8 cores x 4 samples, maker params replicated.
"""

import sys

import numpy as np

sys.path.insert(0, "/opt/trn_rl_repo")

import concourse.bacc as bacc  # noqa: E402
import concourse.tile as tile  # noqa: E402
from concourse import mybir  # noqa: E402
from concourse.bass_utils import run_bass_kernel_spmd  # noqa: E402

import ml_dtypes  # noqa: E402

B, C_IN, C_OUT, K, L = 32, 64, 128, 3, 8192
N_CORES = 8
BS = B // N_CORES          # samples per core
NT = 512                   # matmul moving-dim tile (one PSUM bank of fp32)
NTILES = L // NT
NCH = 4                    # x-load / shifted-copy / partial-reduce chunks
CHW = (L + 2) // NCH       # 2048, last chunk takes the +2 remainder

F32 = mybir.dt.float32
F32R = mybir.dt.float32r
BF16 = mybir.dt.bfloat16


def _host_precompute(W_kernel, W_in, W_out, W_bias):
    """Fold the maker parameters into linear maps on the stats (S, E, F)."""
    Wk = W_kernel.reshape(C_IN, K, K).astype(np.float64)     # [c, j, t]
    P = (Wk[:, :, 0] + Wk[:, :, 1] + Wk[:, :, 2]) / L        # coeff on S
    Q = -Wk[:, :, 0] / L                                     # coeff on E
    R = -Wk[:, :, 2] / L                                     # coeff on F

    Win = W_in[:, :, 0].astype(np.float64)                   # [c, c']

    def m_in(Xc):   # -> [c', k*64+c]
        return np.einsum("cp,pk->pkc", Win, Xc).reshape(C_IN, K * C_IN)

    def m_out(Xc, W):  # -> [c', o]
        return np.einsum("ock,ck->co", W.astype(np.float64), Xc)

    def mm(Xc):
        return np.concatenate([m_in(Xc), m_out(Xc, W_out)], axis=1)  # [64,320]

    m3 = np.stack([mm(P), mm(Q), mm(R)], axis=1)             # [64, 3, 320]
    mb3 = np.stack(
        [m_out(P, W_bias), m_out(Q, W_bias), m_out(R, W_bias)], axis=1
    )                                                        # [64, 3, 128]
    return m3.astype(np.float32), mb3.astype(np.float32)


_CACHE = {}


def _emit_stage(nc, xp, small, x_d, b):
    """Issue x load, shifted copy, and chunked stats for sample b.

    Loads and shifted copies use two big chunks (8KB per-partition rows:
    DMA queues are descriptor-rate limited, so fewer/fatter descriptors).
    Stats use four windows aligned to the load halves so each reduce
    depends on exactly one load chunk.
    """
    H = 4097                                 # load-chunk boundary
    xh = xp.tile([2 * C_IN, L + 2], BF16, tag="xh")
    Sp = small.tile([C_IN, NCH], F32, tag="Sp")
    nc.sync.dma_start(xh[0:C_IN, 0:H], x_d[b][:, 0:H])
    nc.sync.dma_start(xh[0:C_IN, H:L + 2], x_d[b][:, H:L + 2])
    # dest col j <- src col j+1 (chunk c depends only on load chunk c)
    nc.sync.dma_start(xh[C_IN:, 0:H - 1], xh[0:C_IN, 1:H])
    nc.sync.dma_start(xh[C_IN:, H - 1:L + 1], xh[0:C_IN, H:L + 2])
    # both zero-pad columns are included: they add nothing to S
    for c, (c0, c1) in enumerate(((0, 2048), (2048, H),
                                  (H, 6144), (6144, L + 2))):
        nc.vector.reduce_sum(out=Sp[:, c:c + 1], in_=xh[0:C_IN, c0:c1],
                             axis=mybir.AxisListType.X)
    return xh, Sp


def _emit_synth_steps(nc, small, pss, m3, mb3, xh, Sp):
    """Stats -> (w01, w2, biasv) for one sample, as four deferred steps.

    The steps are interleaved into the PREVIOUS sample's conv matmul
    stream so the PE <-> DVE ping-pong never drains the tensor engine
    (which would also drop its p-state).  Synth PSUM packs into two
    banks (disjoint address ranges, so interleaved accumulation groups
    are safe: skip_group_check).
    """
    stat = small.tile([C_IN, 3], F32R, tag="stat")
    syn_pb = pss.tile([C_OUT, 512], F32, tag="syn_pb")
    syn_w = pss.tile([2 * C_IN, 256], F32, tag="syn_w")
    psp, psb = syn_pb[0:1, 0:320], syn_pb[:, 320:321]
    ps01, ps2 = syn_w[:, 0:128], syn_w[C_IN:, 128:256]
    params = small.tile([1, 320], F32R, tag="params")
    biasv = small.tile([C_OUT, 1], F32, tag="biasv")
    w01 = small.tile([2 * C_IN, C_OUT], BF16, tag="w01")
    w2 = small.tile([2 * C_IN, C_OUT], BF16, tag="w2")

    def step0():   # stats gather (DVE) + stat matmuls (PE)
        # fp32r is 32-bit in SBUF: the low-precision guard is a false alarm
        with nc.allow_low_precision(reason="fp32r out is fp32 bits"):
            nc.vector.reduce_sum(out=stat[:, 0:1], in_=Sp[:],
                                 axis=mybir.AxisListType.X)
        nc.vector.tensor_copy(stat[:, 1:2], xh[0:C_IN, L:L + 1])   # E
        nc.vector.tensor_copy(stat[:, 2:3], xh[0:C_IN, 1:2])       # F
        for j in range(3):
            sj = stat[:, j:j + 1]
            nc.tensor.matmul(psp, sj, m3[:, j, :], start=(j == 0),
                             stop=(j == 2), skip_group_check=True)
            # 1 moving column: fp32 4-pass costs nothing, and fp32r
            # moving free-size 1 fails the ISA check
            nc.tensor.matmul(psb, mb3[:, j, :], sj.bitcast(F32),
                             start=(j == 0), stop=(j == 2),
                             skip_group_check=True)

    def step1():
        nc.vector.tensor_copy(params[:], psp)
        nc.vector.tensor_copy(biasv[:], psb)

    def step2():
        # rank-1 stationaries: [W0; W1] on partitions 0-127, W2 on
        # 64-127.  contract-1 outers: fp32r fails the ISA check, fp32
        # 4-pass on 128 moving cols is well under a microsecond.
        pr = params[0:1].bitcast(F32)
        w_out_row = pr[:, 192:320]
        nc.tensor.matmul(ps01, pr[:, 0:128], w_out_row, start=True,
                         stop=True, skip_group_check=True)
        nc.tensor.matmul(ps2, pr[:, 128:192], w_out_row, start=True,
                         stop=True, skip_group_check=True)

    def step3():
        nc.vector.tensor_copy(w01[:], ps01)
        nc.vector.tensor_copy(w2[C_IN:, :], ps2)

    return (w01, w2, biasv), [step0, step1, step2, step3]


def _emit_conv(nc, yp, psy, y_d, b, xh, w01, w2, biasv, steps=()):
    """Main conv for one sample: 16 tiles x (2 matmuls, evict); 2048-col
    store chunks.  `steps` are the next sample's synth stages, dropped
    into the instruction stream mid-conv."""
    SCW = 4 * NT                             # store-chunk columns
    ysb = None
    step_at = {3: 0, 6: 1, 9: 2, 12: 3}
    for t in range(NTILES):
        if t % 4 == 0:
            ysb = yp.tile([C_OUT, SCW], F32, tag="ysb")
        py = psy.tile([C_OUT, NT], F32, tag="py")
        m = NT * t
        nc.tensor.matmul(py[:], w01[:], xh[:, m:m + NT],
                         start=True, stop=False)
        nc.tensor.matmul(py[:], w2[C_IN:, :], xh[C_IN:, m + 1:m + NT + 1],
                         start=False, stop=True)
        off = (t % 4) * NT
        if t % 2 == 0:
            nc.scalar.activation(ysb[:, off:off + NT], py[:],
                                 mybir.ActivationFunctionType.Identity,
                                 bias=biasv[:], scale=1.0)
        else:
            nc.vector.tensor_scalar(out=ysb[:, off:off + NT], in0=py[:],
                                    scalar1=biasv[:], scalar2=None,
                                    op0=mybir.AluOpType.add)
            if t % 4 == 3:
                c0 = (t - 3) * NT
                nc.sync.dma_start(y_d[b][:, c0:c0 + SCW], ysb[:])
        if t in step_at and step_at[t] < len(steps):
            steps[step_at[t]]()


def _build_module():
    if "nc" in _CACHE:
        return _CACHE["nc"]
    nc = bacc.Bacc("TRN2", target_bir_lowering=False, debug=False)

    # host supplies x pre-padded with one zero column on each side, bf16
    x_d = nc.dram_tensor("x", [BS, C_IN, L + 2], BF16,
                         kind="ExternalInput").ap()
    m3_d = nc.dram_tensor("m3", [C_IN, 3, 320], F32R,
                          kind="ExternalInput").ap()
    mb3_d = nc.dram_tensor("mb3", [C_IN, 3, C_OUT], F32,
                           kind="ExternalInput").ap()
    y_d = nc.dram_tensor("y", [BS, C_OUT, L], F32,
                         kind="ExternalOutput").ap()

    with tile.TileContext(nc) as tc:
        with (
            tc.tile_pool(name="consts", bufs=1) as consts,
            tc.tile_pool(name="xp", bufs=4) as xp,
            tc.tile_pool(name="yp", bufs=4) as yp,
            tc.tile_pool(name="small", bufs=2) as small,
            tc.tile_pool(name="ps_y", bufs=6, space="PSUM") as psy,
            tc.tile_pool(name="ps_s", bufs=1, space="PSUM") as pss,
        ):
            m3 = consts.tile([C_IN, 3, 320], F32R)
            mb3 = consts.tile([C_IN, 3, C_OUT], F32)
            nc.sync.dma_start(m3[:], m3_d)
            nc.sync.dma_start(mb3[:], mb3_d)

            # software pipeline: stage(b) issues loads/copies/stats, synth(b)
            # runs the small fp32r matmul chain, conv(b) the 16-tile conv.
            # stage(b+2) is issued before conv(b) so its DMAs sit ahead of
            # conv(b)'s stores in the queues; stats(b+1)/(b+2) sit ahead of
            # conv(b)'s DVE evictions.
            stages = {}
            stages[0] = _emit_stage(nc, xp, small, x_d, 0)
            tiles0, steps0 = _emit_synth_steps(nc, small, pss, m3, mb3,
                                               *stages[0])
            for s in steps0:           # sample 0: run synth immediately
                s()
            stages[1] = _emit_stage(nc, xp, small, x_d, 1)
            synth = {0: tiles0}
            nxt = {}
            for b in range(BS):
                if b + 2 < BS:
                    stages[b + 2] = _emit_stage(nc, xp, small, x_d, b + 2)
                if b + 1 < BS:
                    synth[b + 1], nxt_steps = _emit_synth_steps(
                        nc, small, pss, m3, mb3, *stages[b + 1])
                else:
                    nxt_steps = ()
                _emit_conv(nc, yp, psy, y_d, b, stages[b][0], *synth[b],
                           steps=nxt_steps)

    nc.compile()
    _CACHE["nc"] = nc
    return nc


def kernel(x, W_kernel, W_in, W_out, W_bias):
    x = np.asarray(x, dtype=np.float32)
    # one zero column each side: the device reads x[l-1], x[l], x[l+1]
    x = np.pad(x, [(0, 0), (0, 0), (1, 1)]).astype(ml_dtypes.bfloat16)
    m3, mb3 = _host_precompute(
        np.asarray(W_kernel, np.float32), np.asarray(W_in, np.float32),
        np.asarray(W_out, np.float32), np.asarray(W_bias, np.float32))

    nc = _build_module()
    in_maps = [
        {"x": x[c * BS:(c + 1) * BS], "m3": m3, "mb3": mb3}
        for c in range(N_CORES)
    ]
    res = run_bass_kernel_spmd(nc, in_maps, core_ids=list(range(N_CORES)))
    global LAST_RESULT
    LAST_RESULT = res
    y = np.concatenate([r["y"] for r in res.results], axis=0)
    return y


LAST_RESULT = None


# revision 10
# speedup vs baseline: 1.0619x; 1.0619x over previous
"""Trainium2 Bass kernel for nn_ConvPlus1d (dense_cnn).

Algorithm (mathematically identical to the reference, derived analytically):

  The reference synthesizes per-sample conv weights:
      kern[b]   = mean_L(depthwise_conv(x))        -> [B, C_IN, K]
      w_in[b]   = W_in @ kern[b]                   -> [B, C_IN, K]
      w_out[b]  = <W_out, kern[b]>                 -> [B, C_OUT]
      bias[b]   = <W_bias, kern[b]>                -> [B, C_OUT]
      weight[b, o, c, k] = w_in[b, c, k] * w_out[b, o]     (rank-1!)
      y[b] = conv1d(x[b], weight[b], pad=1) + bias[b]

  Exact simplifications:
  1) mean over L of a pad-1 depthwise conv only needs per-channel sums and
     the first/last elements:  sum_l xpad[c, l+t] = {S-E, S, S-F}[t]
     so kern / w_in / w_out / bias are LINEAR in (S, E, F), with
     coefficient matrices precomputed on the host from maker params.
  2) The per-sample conv weight is rank-1 across (o) x (c,k).

  Device program per sample (data-parallel over batch, 4 samples/core):
      x (bf16) lands in SBUF partitions 0-63; a shifted copy (one column
      left) is DMA'd into partitions 64-127.  The 3-tap conv then needs
      only TWO matmuls per 512-col tile: a 128-contract matmul computes
      taps 0+1 together (stationary [W0; W1]), a 64-contract matmul adds
      tap 2.  Stats -> params synthesis runs in fp32r (1 cyc/row).
      PSUM -> SBUF eviction adds the bias (ACT/DVE alternating); stores
      stream out in 1024-col chunks.

Sharding: batch 32 -> 8 cores x 4 samples, maker params replicated.
"""

import sys

import numpy as np

sys.path.insert(0, "/opt/trn_rl_repo")

import concourse.bacc as bacc  # noqa: E402
import concourse.tile as tile  # noqa: E402
from concourse import mybir  # noqa: E402
from concourse.bass_utils import run_bass_kernel_spmd  # noqa: E402

import ml_dtypes  # noqa: E402

B, C_IN, C_OUT, K, L = 32, 64, 128, 3, 8192
N_CORES = 8
BS = B // N_CORES          # samples per core
NT = 512                   # matmul moving-dim tile (one PSUM bank of fp32)
NTILES = L // NT
NCH = 4                    # x-load / shifted-copy / partial-reduce chunks
CHW = (L + 2) // NCH       # 2048, last chunk takes the +2 remainder

F32 = mybir.dt.float32
F32R = mybir.dt.float32r
BF16 = mybir.dt.bfloat16


def _host_precompute(W_kernel, W_in, W_out, W_bias):
    """Fold the maker parameters into linear maps on the stats (S, E, F)."""
    Wk = W_kernel.reshape(C_IN, K, K).astype(np.float64)     # [c, j, t]
    P = (Wk[:, :, 0] + Wk[:, :, 1] + Wk[:, :, 2]) / L        # coeff on S
    Q = -Wk[:, :, 0] / L                                     # coeff on E
    R = -Wk[:, :, 2] / L                                     # coeff on F

    Win = W_in[:, :, 0].astype(np.float64)                   # [c, c']

    def m_in(Xc):   # -> [c', k*64+c]
        return np.einsum("cp,pk->pkc", Win, Xc).reshape(C_IN, K * C_IN)

    def m_out(Xc, W):  # -> [c', o]
        return np.einsum("ock,ck->co", W.astype(np.float64), Xc)

    def mm(Xc):
        return np.concatenate([m_in(Xc), m_out(Xc, W_out)], axis=1)  # [64,320]

    m3 = np.stack([mm(P), mm(Q), mm(R)], axis=1)             # [64, 3, 320]
    mb3 = np.stack(
        [m_out(P, W_bias), m_out(Q, W_bias), m_out(R, W_bias)], axis=1
    )                                                        # [64, 3, 128]
    return m3.astype(np.float32), mb3.astype(np.float32)


_CACHE = {}


def _emit_stage(nc, xp, small, x_d, b):
    """Issue x load, shifted copy, and chunked stats for sample b.

    Loads and shifted copies use two big chunks (8KB per-partition rows:
    DMA queues are descriptor-rate limited, so fewer/fatter descriptors).
    Stats use four windows aligned to the load halves so each reduce
    depends on exactly one load chunk.
    """
    H = 4097                                 # load-chunk boundary
    xh = xp.tile([2 * C_IN, L + 2], BF16, tag="xh")
    Sp = small.tile([C_IN, NCH], F32, tag="Sp")
    nc.sync.dma_start(xh[0:C_IN, 0:H], x_d[b][:, 0:H])
    nc.sync.dma_start(xh[0:C_IN, H:L + 2], x_d[b][:, H:L + 2])
    # dest col j <- src col j+1 (chunk c depends only on load chunk c)
    nc.sync.dma_start(xh[C_IN:, 0:H - 1], xh[0:C_IN, 1:H])
    nc.sync.dma_start(xh[C_IN:, H - 1:L + 1], xh[0:C_IN, H:L + 2])
    # both zero-pad columns are included: they add nothing to S
    for c, (c0, c1) in enumerate(((0, 2048), (2048, H),
                                  (H, 6144), (6144, L + 2))):
        nc.vector.reduce_sum(out=Sp[:, c:c + 1], in_=xh[0:C_IN, c0:c1],
                             axis=mybir.AxisListType.X)
    return xh, Sp


def _emit_synth_steps(nc, small, pss, m3, mb3, xh, Sp):
    """Stats -> (w01, w2, biasv) for one sample, as four deferred steps.

    The steps are interleaved into the PREVIOUS sample's conv matmul
    stream so the PE <-> DVE ping-pong never drains the tensor engine
    (which would also drop its p-state).  Synth PSUM packs into two
    banks (disjoint address ranges, so interleaved accumulation groups
    are safe: skip_group_check).
    """
    stat = small.tile([C_IN, 3], F32R, tag="stat")
    syn_pb = pss.tile([C_OUT, 512], F32, tag="syn_pb")
    syn_w = pss.tile([2 * C_IN, 256], F32, tag="syn_w")
    psp, psb = syn_pb[0:1, 0:320], syn_pb[:, 320:321]
    ps01, ps2 = syn_w[:, 0:128], syn_w[C_IN:, 128:256]
    params = small.tile([1, 320], F32R, tag="params")
    biasv = small.tile([C_OUT, 1], F32, tag="biasv")
    w01 = small.tile([2 * C_IN, C_OUT], BF16, tag="w01")
    w2 = small.tile([2 * C_IN, C_OUT], BF16, tag="w2")

    def step0():   # stats gather (DVE) + stat matmuls (PE)
        # fp32r is 32-bit in SBUF: the low-precision guard is a false alarm
        with nc.allow_low_precision(reason="fp32r out is fp32 bits"):
            nc.vector.reduce_sum(out=stat[:, 0:1], in_=Sp[:],
                                 axis=mybir.AxisListType.X)
        nc.vector.tensor_copy(stat[:, 1:2], xh[0:C_IN, L:L + 1])   # E
        nc.vector.tensor_copy(stat[:, 2:3], xh[0:C_IN, 1:2])       # F
        for j in range(3):
            sj = stat[:, j:j + 1]
            nc.tensor.matmul(psp, sj, m3[:, j, :], start=(j == 0),
                             stop=(j == 2), skip_group_check=True)
            # 1 moving column: fp32 4-pass costs nothing, and fp32r
            # moving free-size 1 fails the ISA check
            nc.tensor.matmul(psb, mb3[:, j, :], sj.bitcast(F32),
                             start=(j == 0), stop=(j == 2),
                             skip_group_check=True)

    def step1():
        nc.vector.tensor_copy(params[:], psp)
        nc.vector.tensor_copy(biasv[:], psb)

    def step2():
        # rank-1 stationaries: [W0; W1] on partitions 0-127, W2 on
        # 64-127.  contract-1 outers: fp32r fails the ISA check, fp32
        # 4-pass on 128 moving cols is well under a microsecond.
        pr = params[0:1].bitcast(F32)
        w_out_row = pr[:, 192:320]
        nc.tensor.matmul(ps01, pr[:, 0:128], w_out_row, start=True,
                         stop=True, skip_group_check=True)
        nc.tensor.matmul(ps2, pr[:, 128:192], w_out_row, start=True,
                         stop=True, skip_group_check=True)

    def step3():
        nc.vector.tensor_copy(w01[:], ps01)
        nc.vector.tensor_copy(w2[C_IN:, :], ps2)

    return (w01, w2, biasv), [step0, step1, step2, step3]


def _emit_conv(nc, yp, psy, y_d, b, xh, w01, w2, biasv, steps=()):
    """Main conv for one sample: 16 tiles x (2 matmuls, evict); 2048-col
    store chunks.  `steps` are the next sample's synth stages, dropped
    into the instruction stream mid-conv."""
    SCW = 4 * NT                             # store-chunk columns
    ysb = None
    step_at = {3: 0, 6: 1, 9: 2, 12: 3}
    for t in range(NTILES):
        if t % 4 == 0:
            ysb = yp.tile([C_OUT, SCW], F32, tag="ysb")
        py = psy.tile([C_OUT, NT], F32, tag="py")
        m = NT * t
        nc.tensor.matmul(py[:], w01[:], xh[:, m:m + NT],
                         start=True, stop=False)
        nc.tensor.matmul(py[:], w2[C_IN:, :], xh[C_IN:, m + 1:m + NT + 1],
                         start=False, stop=True)
        off = (t % 4) * NT
        # evictions live on ACT alone: DVE holds the 8.8us/sample stats
        # reduces, and any eviction queued behind them stalls the PE on
        # PSUM banks.  ACT evicts in ~0.69us vs the 0.88us bank period.
        nc.scalar.activation(ysb[:, off:off + NT], py[:],
                             mybir.ActivationFunctionType.Identity,
                             bias=biasv[:], scale=1.0)
        if t % 4 == 3:
            c0 = (t - 3) * NT
            nc.sync.dma_start(y_d[b][:, c0:c0 + SCW], ysb[:])
        if t in step_at and step_at[t] < len(steps):
            steps[step_at[t]]()


def _build_module():
    if "nc" in _CACHE:
        return _CACHE["nc"]
    nc = bacc.Bacc("TRN2", target_bir_lowering=False, debug=False)

    # host supplies x pre-padded with one zero column on each side, bf16
    x_d = nc.dram_tensor("x", [BS, C_IN, L + 2], BF16,
                         kind="ExternalInput").ap()
    m3_d = nc.dram_tensor("m3", [C_IN, 3, 320], F32R,
                          kind="ExternalInput").ap()
    mb3_d = nc.dram_tensor("mb3", [C_IN, 3, C_OUT], F32,
                           kind="ExternalInput").ap()
    y_d = nc.dram_tensor("y", [BS, C_OUT, L], F32,
                         kind="ExternalOutput").ap()

    with tile.TileContext(nc) as tc:
        with (
            tc.tile_pool(name="consts", bufs=1) as consts,
            tc.tile_pool(name="xp", bufs=4) as xp,
            tc.tile_pool(name="yp", bufs=4) as yp,
            tc.tile_pool(name="small", bufs=2) as small,
            tc.tile_pool(name="ps_y", bufs=6, space="PSUM") as psy,
            tc.tile_pool(name="ps_s", bufs=1, space="PSUM") as pss,
        ):
            m3 = consts.tile([C_IN, 3, 320], F32R)
            mb3 = consts.tile([C_IN, 3, C_OUT], F32)
            nc.sync.dma_start(m3[:], m3_d)
            nc.sync.dma_start(mb3[:], mb3_d)

            # software pipeline: stage(b) issues loads/copies/stats, synth(b)
            # runs the small fp32r matmul chain, conv(b) the 16-tile conv.
            # stage(b+2) is issued before conv(b) so its DMAs sit ahead of
            # conv(b)'s stores in the queues; stats(b+1)/(b+2) sit ahead of
            # conv(b)'s DVE evictions.
            stages = {}
            stages[0] = _emit_stage(nc, xp, small, x_d, 0)
            tiles0, steps0 = _emit_synth_steps(nc, small, pss, m3, mb3,
                                               *stages[0])
            for s in steps0:           # sample 0: run synth immediately
                s()
            stages[1] = _emit_stage(nc, xp, small, x_d, 1)
            synth = {0: tiles0}
            nxt = {}
            for b in range(BS):
                if b + 2 < BS:
                    stages[b + 2] = _emit_stage(nc, xp, small, x_d, b + 2)
                if b + 1 < BS:
                    synth[b + 1], nxt_steps = _emit_synth_steps(
                        nc, small, pss, m3, mb3, *stages[b + 1])
                else:
                    nxt_steps = ()
                _emit_conv(nc, yp, psy, y_d, b, stages[b][0], *synth[b],
                           steps=nxt_steps)

    nc.compile()
    _CACHE["nc"] = nc
    return nc


def kernel(x, W_kernel, W_in, W_out, W_bias):
    x = np.asarray(x, dtype=np.float32)
    # one zero column each side: the device reads x[l-1], x[l], x[l+1]
    x = np.pad(x, [(0, 0), (0, 0), (1, 1)]).astype(ml_dtypes.bfloat16)
    m3, mb3 = _host_precompute(
        np.asarray(W_kernel, np.float32), np.asarray(W_in, np.float32),
        np.asarray(W_out, np.float32), np.asarray(W_bias, np.float32))

    nc = _build_module()
    in_maps = [
        {"x": x[c * BS:(c + 1) * BS], "m3": m3, "mb3": mb3}
        for c in range(N_CORES)
    ]
    res = run_bass_kernel_spmd(nc, in_maps, core_ids=list(range(N_CORES)))
    global LAST_RESULT
    LAST_RESULT = res
    y = np.concatenate([r["y"] for r in res.results], axis=0)
    return y


LAST_RESULT = None


# revision 11
# speedup vs baseline: 1.0827x; 1.0197x over previous
"""Trainium2 Bass kernel for nn_ConvPlus1d (dense_cnn).

Algorithm (mathematically identical to the reference, derived analytically):

  The reference synthesizes per-sample conv weights:
      kern[b]   = mean_L(depthwise_conv(x))        -> [B, C_IN, K]
      w_in[b]   = W_in @ kern[b]                   -> [B, C_IN, K]
      w_out[b]  = <W_out, kern[b]>                 -> [B, C_OUT]
      bias[b]   = <W_bias, kern[b]>                -> [B, C_OUT]
      weight[b, o, c, k] = w_in[b, c, k] * w_out[b, o]     (rank-1!)
      y[b] = conv1d(x[b], weight[b], pad=1) + bias[b]

  Exact simplifications:
  1) mean over L of a pad-1 depthwise conv only needs per-channel sums and
     the first/last elements:  sum_l xpad[c, l+t] = {S-E, S, S-F}[t]
     so kern / w_in / w_out / bias are LINEAR in (S, E, F), with
     coefficient matrices precomputed on the host from maker params.
  2) The per-sample conv weight is rank-1 across (o) x (c,k).

  Device program per sample (data-parallel over batch, 4 samples/core):
      x (bf16) lands in SBUF partitions 0-63; a shifted copy (one column
      left) is DMA'd into partitions 64-127.  The 3-tap conv then needs
      only TWO matmuls per 512-col tile: a 128-contract matmul computes
      taps 0+1 together (stationary [W0; W1]), a 64-contract matmul adds
      tap 2.  Stats -> params synthesis runs in fp32r (1 cyc/row).
      PSUM -> SBUF eviction adds the bias (ACT/DVE alternating); stores
      stream out in 1024-col chunks.

Sharding: batch 32 -> 8 cores x 4 samples, maker params replicated.
"""

import sys

import numpy as np

sys.path.insert(0, "/opt/trn_rl_repo")

import concourse.bacc as bacc  # noqa: E402
import concourse.tile as tile  # noqa: E402
from concourse import mybir  # noqa: E402
from concourse.bass_utils import run_bass_kernel_spmd  # noqa: E402

import ml_dtypes  # noqa: E402

B, C_IN, C_OUT, K, L = 32, 64, 128, 3, 8192
N_CORES = 8
BS = B // N_CORES          # samples per core
NT = 512                   # matmul moving-dim tile (one PSUM bank of fp32)
NTILES = L // NT
NCH = 8                    # partial-reduce windows
CHW = (L + 2) // NCH       # 2048, last chunk takes the +2 remainder

F32 = mybir.dt.float32
F32R = mybir.dt.float32r
BF16 = mybir.dt.bfloat16


def _host_precompute(W_kernel, W_in, W_out, W_bias):
    """Fold the maker parameters into linear maps on the stats (S, E, F)."""
    Wk = W_kernel.reshape(C_IN, K, K).astype(np.float64)     # [c, j, t]
    P = (Wk[:, :, 0] + Wk[:, :, 1] + Wk[:, :, 2]) / L        # coeff on S
    Q = -Wk[:, :, 0] / L                                     # coeff on E
    R = -Wk[:, :, 2] / L                                     # coeff on F

    Win = W_in[:, :, 0].astype(np.float64)                   # [c, c']

    def m_in(Xc):   # -> [c', k*64+c]
        return np.einsum("cp,pk->pkc", Win, Xc).reshape(C_IN, K * C_IN)

    def m_out(Xc, W):  # -> [c', o]
        return np.einsum("ock,ck->co", W.astype(np.float64), Xc)

    def mm(Xc):
        return np.concatenate([m_in(Xc), m_out(Xc, W_out)], axis=1)  # [64,320]

    m3 = np.stack([mm(P), mm(Q), mm(R)], axis=1)             # [64, 3, 320]
    mb3 = np.stack(
        [m_out(P, W_bias), m_out(Q, W_bias), m_out(R, W_bias)], axis=1
    )                                                        # [64, 3, 128]
    return m3.astype(np.float32), mb3.astype(np.float32)


_CACHE = {}


def _emit_stage(nc, xp, small, x_d, b):
    """Issue x load, shifted copy, and chunked stats for sample b.

    Loads and shifted copies use two big chunks (8KB per-partition rows:
    DMA queues are descriptor-rate limited, so fewer/fatter descriptors).
    Stats use four windows aligned to the load halves so each reduce
    depends on exactly one load chunk.
    """
    H = 4097                                 # load-chunk boundary
    xh = xp.tile([2 * C_IN, L + 2], BF16, tag="xh")
    Sp = small.tile([C_IN, NCH], F32, tag="Sp")
    nc.sync.dma_start(xh[0:C_IN, 0:H], x_d[b][:, 0:H])
    nc.sync.dma_start(xh[0:C_IN, H:L + 2], x_d[b][:, H:L + 2])
    # dest col j <- src col j+1 (chunk c depends only on load chunk c)
    nc.sync.dma_start(xh[C_IN:, 0:H - 1], xh[0:C_IN, 1:H])
    nc.sync.dma_start(xh[C_IN:, H - 1:L + 1], xh[0:C_IN, H:L + 2])
    # both zero-pad columns are included: they add nothing to S.
    # 8 windows aligned to the load halves: each reduce starts as soon
    # as its half has landed, hiding the DVE serial time behind the DMA.
    bnds = (0, 1024, 2048, 3072, H, 5169, 6241, 7313, L + 2)
    for c in range(8):
        nc.vector.reduce_sum(out=Sp[:, c:c + 1],
                             in_=xh[0:C_IN, bnds[c]:bnds[c + 1]],
                             axis=mybir.AxisListType.X)
    return xh, Sp


def _emit_synth_steps(nc, small, pss, m3, mb3, xh, Sp):
    """Stats -> (w01, w2, biasv) for one sample, as four deferred steps.

    The steps are interleaved into the PREVIOUS sample's conv matmul
    stream so the PE <-> DVE ping-pong never drains the tensor engine
    (which would also drop its p-state).  Synth PSUM packs into two
    banks (disjoint address ranges, so interleaved accumulation groups
    are safe: skip_group_check).
    """
    stat = small.tile([C_IN, 3], F32R, tag="stat")
    syn_pb = pss.tile([C_OUT, 512], F32, tag="syn_pb")
    syn_w = pss.tile([2 * C_IN, 256], F32, tag="syn_w")
    psp, psb = syn_pb[0:1, 0:320], syn_pb[:, 320:321]
    ps01, ps2 = syn_w[:, 0:128], syn_w[C_IN:, 128:256]
    params = small.tile([1, 320], F32R, tag="params")
    biasv = small.tile([C_OUT, 1], F32, tag="biasv")
    w01 = small.tile([2 * C_IN, C_OUT], BF16, tag="w01")
    w2 = small.tile([2 * C_IN, C_OUT], BF16, tag="w2")

    def step0():   # stats gather (DVE) + stat matmuls (PE)
        # fp32r is 32-bit in SBUF: the low-precision guard is a false alarm
        with nc.allow_low_precision(reason="fp32r out is fp32 bits"):
            nc.vector.reduce_sum(out=stat[:, 0:1], in_=Sp[:],
                                 axis=mybir.AxisListType.X)
        nc.vector.tensor_copy(stat[:, 1:2], xh[0:C_IN, L:L + 1])   # E
        nc.vector.tensor_copy(stat[:, 2:3], xh[0:C_IN, 1:2])       # F
        for j in range(3):
            sj = stat[:, j:j + 1]
            nc.tensor.matmul(psp, sj, m3[:, j, :], start=(j == 0),
                             stop=(j == 2), skip_group_check=True)
            # 1 moving column: fp32 4-pass costs nothing, and fp32r
            # moving free-size 1 fails the ISA check
            nc.tensor.matmul(psb, mb3[:, j, :], sj.bitcast(F32),
                             start=(j == 0), stop=(j == 2),
                             skip_group_check=True)

    def step1():
        nc.vector.tensor_copy(params[:], psp)
        nc.vector.tensor_copy(biasv[:], psb)

    def step2():
        # rank-1 stationaries: [W0; W1] on partitions 0-127, W2 on
        # 64-127.  contract-1 outers: fp32r fails the ISA check, fp32
        # 4-pass on 128 moving cols is well under a microsecond.
        pr = params[0:1].bitcast(F32)
        w_out_row = pr[:, 192:320]
        nc.tensor.matmul(ps01, pr[:, 0:128], w_out_row, start=True,
                         stop=True, skip_group_check=True)
        nc.tensor.matmul(ps2, pr[:, 128:192], w_out_row, start=True,
                         stop=True, skip_group_check=True)

    def step3():
        nc.vector.tensor_copy(w01[:], ps01)
        nc.vector.tensor_copy(w2[C_IN:, :], ps2)

    return (w01, w2, biasv), [step0, step1, step2, step3]


def _emit_conv(nc, yp, psy, y_d, b, xh, w01, w2, biasv, steps=()):
    """Main conv for one sample: 16 tiles x (2 matmuls, evict); 2048-col
    store chunks.  `steps` are the next sample's synth stages, dropped
    into the instruction stream mid-conv."""
    SCW = 4 * NT                             # store-chunk columns
    ysb = None
    step_at = {3: 0, 6: 1, 9: 2, 12: 3}
    for t in range(NTILES):
        if t % 4 == 0:
            ysb = yp.tile([C_OUT, SCW], F32, tag="ysb")
        py = psy.tile([C_OUT, NT], F32, tag="py")
        m = NT * t
        nc.tensor.matmul(py[:], w01[:], xh[:, m:m + NT],
                         start=True, stop=False)
        nc.tensor.matmul(py[:], w2[C_IN:, :], xh[C_IN:, m + 1:m + NT + 1],
                         start=False, stop=True)
        off = (t % 4) * NT
        # evictions live on ACT alone: DVE holds the 8.8us/sample stats
        # reduces, and any eviction queued behind them stalls the PE on
        # PSUM banks.  ACT evicts in ~0.69us vs the 0.88us bank period.
        nc.scalar.activation(ysb[:, off:off + NT], py[:],
                             mybir.ActivationFunctionType.Identity,
                             bias=biasv[:], scale=1.0)
        if t % 4 == 3:
            c0 = (t - 3) * NT
            nc.sync.dma_start(y_d[b][:, c0:c0 + SCW], ysb[:])
        if t in step_at and step_at[t] < len(steps):
            steps[step_at[t]]()


def _build_module():
    if "nc" in _CACHE:
        return _CACHE["nc"]
    nc = bacc.Bacc("TRN2", target_bir_lowering=False, debug=False)

    # host supplies x pre-padded with one zero column on each side, bf16
    x_d = nc.dram_tensor("x", [BS, C_IN, L + 2], BF16,
                         kind="ExternalInput").ap()
    m3_d = nc.dram_tensor("m3", [C_IN, 3, 320], F32R,
                          kind="ExternalInput").ap()
    mb3_d = nc.dram_tensor("mb3", [C_IN, 3, C_OUT], F32,
                           kind="ExternalInput").ap()
    y_d = nc.dram_tensor("y", [BS, C_OUT, L], F32,
                         kind="ExternalOutput").ap()

    with tile.TileContext(nc) as tc:
        with (
            tc.tile_pool(name="consts", bufs=1) as consts,
            tc.tile_pool(name="xp", bufs=4) as xp,
            tc.tile_pool(name="yp", bufs=4) as yp,
            tc.tile_pool(name="small", bufs=2) as small,
            tc.tile_pool(name="ps_y", bufs=6, space="PSUM") as psy,
            tc.tile_pool(name="ps_s", bufs=1, space="PSUM") as pss,
        ):
            m3 = consts.tile([C_IN, 3, 320], F32R)
            mb3 = consts.tile([C_IN, 3, C_OUT], F32)

            # software pipeline: stage(b) issues loads/copies/stats, synth(b)
            # runs the small fp32r matmul chain, conv(b) the 16-tile conv.
            # stage(b+2) is issued before conv(b) so its DMAs sit ahead of
            # conv(b)'s stores in the queues; stats(b+1)/(b+2) sit ahead of
            # conv(b)'s DVE evictions.  Sample 0's x load is issued before
            # the consts so it is the first transfer in the queues; consts
            # trigger from the Scalar engine to spread descriptor-gen.
            stages = {}
            stages[0] = _emit_stage(nc, xp, small, x_d, 0)
            nc.scalar.dma_start(m3[:], m3_d)
            nc.scalar.dma_start(mb3[:], mb3_d)
            tiles0, steps0 = _emit_synth_steps(nc, small, pss, m3, mb3,
                                               *stages[0])
            for s in steps0:           # sample 0: run synth immediately
                s()
            stages[1] = _emit_stage(nc, xp, small, x_d, 1)
            synth = {0: tiles0}
            nxt = {}
            for b in range(BS):
                if b + 2 < BS:
                    stages[b + 2] = _emit_stage(nc, xp, small, x_d, b + 2)
                if b + 1 < BS:
                    synth[b + 1], nxt_steps = _emit_synth_steps(
                        nc, small, pss, m3, mb3, *stages[b + 1])
                else:
                    nxt_steps = ()
                _emit_conv(nc, yp, psy, y_d, b, stages[b][0], *synth[b],
                           steps=nxt_steps)

    nc.compile()
    _CACHE["nc"] = nc
    return nc


def kernel(x, W_kernel, W_in, W_out, W_bias):
    x = np.asarray(x, dtype=np.float32)
    # one zero column each side: the device reads x[l-1], x[l], x[l+1]
    x = np.pad(x, [(0, 0), (0, 0), (1, 1)]).astype(ml_dtypes.bfloat16)
    m3, mb3 = _host_precompute(
        np.asarray(W_kernel, np.float32), np.asarray(W_in, np.float32),
        np.asarray(W_out, np.float32), np.asarray(W_bias, np.float32))

    nc = _build_module()
    in_maps = [
        {"x": x[c * BS:(c + 1) * BS], "m3": m3, "mb3": mb3}
        for c in range(N_CORES)
    ]
    res = run_bass_kernel_spmd(nc, in_maps, core_ids=list(range(N_CORES)))
    global LAST_RESULT
    LAST_RESULT = res
    y = np.concatenate([r["y"] for r in res.results], axis=0)
    return y


LAST_RESULT = None
